# revision 55
# baseline (speedup 1.0000x reference)
"""Trainium2 Bass kernel for nn_LinkPredictor.

Reference computation (B=4, N=256, T=16, F=128, H=256):
    h = mean_T(nodefeat)                      # [B,N,F]
    a = h @ W1[:, :F].T                       # [B,N,H]
    c = h @ W1[:, F:].T                       # [B,N,H]
    logits[b,i,j] = W2[0] . relu(a[b,i] + c[b,j] + b1) + b2   # [B,N,N]

Sharding: 8 cores; core k handles batch b=k//2, i-half k%2 (128 i-rows x
256 j-cols of one batch's NxN grid).  Each core only needs nodefeat[b].

v8 design (52.7us measured healthy vs 56.4-57.5us v5.3 baseline; known
run-to-run variance ±1.4us from Tile compile-schedule nondeterminism,
and up to +20% when the chip sits in the thermal P-state -- HAM event
duration 4095ns instead of 3413ns.  Timeline at 52.8us: data in by
~13.2 (two 512KB nf DMAs, 4KB descriptors), first act TT ~15.5, act
phase PE-bound at ~109ns/matmul ending ~48, then drain + a fixed ~10us
framework postamble of 255 semaphore clears):
  - The 256 act tiles [128h, 256j] are produced by VE + SE in parallel.
    HW microbench: DVE tensor_scalar with a per-partition scalar POINTER
    is locked to 2x_1p (196ns/tile) -- the ptr occupies the second read
    port -- while immediates reach 4x.  Instead VE produces act tiles in
    BATCHES via tensor_tensor MAX using relu(c+A) = max(c, -A) + A:
    one TT covers nb consecutive i's of one t-half (in0 = cT broadcast
    over i via a stride-0 dim; in1 = pair-duplicated negA2 with a
    stride-1 innermost dim to keep the 2x_1p mode; ~150ns/tile at nb=6).
    The missing + w2.A_i term is a per-row scalar folded into the b2u
    row: b2u[i] = b2 + sum_t mask_t[i] * (w2_t . A_ti), with (w2_t.A_t)
    computed by two 1-col matmuls, masked by a host-baked -mask row
    (VE-covered (i,t) only), then transposed to per-partition form by
    four 1-col matmuls.  The PSUM DRAIN adds it (VE tensor_scalar add /
    SE activation bias, exact f32) -- no closing matmul sits on the
    group-drain critical path; the last act matmul carries stop=True.
  - SE covers the per-block remainder (8-nb tiles) as Relu+bias singles
    from aTb4; a static greedy balances VE/SE clocks (nb ~ 6) including
    the 4 PSUM group drains, which alternate between VE and SE.
  - act loop is r-major/t-inner; per 8-i superblock: TT blocks first,
    then SE singles, then the 16 matmuls in consumption order.  PE is
    the bottleneck at ~109ns/matmul -> ~28.5us pairwise phase.
  - weights packed into one bf16 tensor wb=[w1a|w1c|w2pad] (1 DMA);
    nf split 2+2 across the sync/scalar HWDGE queues; wee/wrow first.
  - cT is built per j-half (pcL from octets 0..15, pcR from 16..31) so
    the pairwise phase starts ~1.5us after the last nf octet lands.
  - wide full-array dummy matmuls during the DMA wait pre-trip the HAM
    clock gate; SE act-table preloaded by a scratch Relu.
"""

import os
import sys

import numpy as np

_B, _N, _T, _F, _H = 4, 256, 16, 128, 256
_NCORES = 8

_CACHE = {}


def _ensure_paths():
    for p in (
        "/root/.axon_site",
        "/root/.axon_site/_ro/trn_rl_repo",
        "/root/.axon_site/_ro/pypackages",
        "/opt/trn_rl_repo",
    ):
        if os.path.isdir(p) and p not in sys.path:
            sys.path.append(p)


# wee param layout (f32 columns; scalar cols 16B-aligned)
_WEE_SMAT = 0   # [0:8) smat
_WEE_B1T0 = 8   # b1 (t=0)
_WEE_B1T1 = 12  # b1 (t=1)
_WEE_NB1T0 = 16  # -b1 (t=0)
_WEE_NB1T1 = 20  # -b1 (t=1)
_WEE_W = 24

# wrow [1, 384] f32: [0:128)=-mask_t0, [128:256)=-mask_t1, [256:384)=b2
_WROW_M0 = 0
_WROW_M1 = 128
_WROW_B2 = 256
_WROW_W = 384

# wb packed weights [128, 2, 323]: [0:128)=w1a.T, [128:256)=w1c.T,
# [256:319)=w2 sliding-window pad (center col 31), [319:323)=smat half
# (t=0 cols hold smat[:,0:4], t=1 cols smat[:,4:8] -- the octet matmul
# streams rhs wb[:, :, 319:323] as 8 cols in j-local order)
_WB_A = 0
_WB_C = 128
_WB_W2 = 256
_WB_SM = 319
_WB_W = 323

_CS = 414.0        # SE activation single [128,256]
_CDRAIN = 400.0    # pl->osb drain


def _tt_cost(nb):
    # VE tensor_tensor bf16 2x_1p, FD = 256*nb
    return (58.0 + 128.0 * nb) / 0.96


_SBW = 8  # superblock width (i's per VE/SE scheduling block)


def _mk_sched():
    """Greedy VE/SE split over 8-i superblocks.  plan[sb][t] = nb
    (leading i's on VE via one TT; the remaining 8-nb are SE singles);
    drains[g] in {'v','s'}.  (16-i blocks with TT12 measured WORSE --
    coarser production granularity beats the per-tile saving.)  SE
    starts with a virtual handicap: its head chain (negA2/aTb4 builds)
    ends ~1.2us after VE is ready, so early superblocks lean on VE."""
    vclk, sclk = 0.0, 1200.0
    plan = [[0, 0] for _ in range(16)]
    drains = []
    for sb in range(16):
        for t in range(2):
            best = None
            for nb in range(_SBW + 1):
                v2 = vclk + (_tt_cost(nb) if nb else 0.0)
                s2 = sclk + _CS * (_SBW - nb)
                m = max(v2, s2)
                if best is None or m < best[0]:
                    best = (m, nb, v2, s2)
            _, nb, vclk, sclk = best
            plan[sb][t] = nb
        if sb % 4 == 3:
            if vclk <= sclk:
                vclk += _CDRAIN
                drains.append("v")
            else:
                sclk += _CDRAIN
                drains.append("s")
    return plan, drains


_PLAN, _DRAINS = _mk_sched()
# mask[t][i] = 1.0 if (i, t) produced via max-form (VE) -> needs +w2.A_i
_MASK = np.zeros((2, 128), dtype=np.float32)
for _sb in range(16):
    for _t in range(2):
        _MASK[_t, _SBW * _sb : _SBW * _sb + _PLAN[_sb][_t]] = 1.0


def build_nc():
    """Build the per-core Bass program (same program for all 8 cores)."""
    _ensure_paths()
    import concourse.mybir as mybir
    import concourse.tile as tile
    from concourse import bacc

    f32 = mybir.dt.float32
    bf16 = mybir.dt.bfloat16
    Alu = mybir.AluOpType
    Act = mybir.ActivationFunctionType

    nc = bacc.Bacc("TRN2", target_bir_lowering=False, debug=False)

    nf = nc.declare_dram_parameter("nf", [128, 32, 128], bf16, isOutput=False)
    wee = nc.declare_dram_parameter("wee", [128, _WEE_W], f32, isOutput=False)
    wrow = nc.declare_dram_parameter("wrow", [1, _WROW_W], f32, isOutput=False)
    wb = nc.declare_dram_parameter("wb", [128, 2, _WB_W], bf16, isOutput=False)
    outd = nc.declare_dram_parameter("out", [4, 32, 256], f32, isOutput=True)

    with tile.TileContext(nc) as tc:
        with (
            tc.tile_pool(name="const", bufs=1) as constp,
            tc.tile_pool(name="data", bufs=1) as datap,
            tc.tile_pool(name="act", bufs=10) as actp,
            tc.tile_pool(name="ph", bufs=1, space="PSUM") as php,
            tc.tile_pool(name="pa", bufs=2, space="PSUM") as pap,
            tc.tile_pool(name="pc", bufs=2, space="PSUM") as pcp,
            tc.tile_pool(name="pl", bufs=3, space="PSUM") as plp,
        ):
            # ---- DMAs: small params first, nf split across both queues ----
            nf_sb = constp.tile([128, 32, 128], bf16, tag="nf")
            wee_sb = constp.tile([128, _WEE_W], f32, tag="wee")
            wrow_sb = constp.tile([1, _WROW_W], f32, tag="wrow")
            wb_sb = constp.tile([128, 2, _WB_W], bf16, tag="wb")
            # The Scalar engine's queue starts ~2.9us late: its auto-inserted
            # ACT_TABLE_LOAD is a multi-us DMA serialized at the queue head.
            # So the early-critical path (wee for smat, octets 0..15) rides
            # the sync queue, and scalar only carries data needed after ~12us
            # (wb, the R-half octets).  nf moves as two 512KB DMAs (4KB
            # contiguous per partition): the 2KB-descriptor version left the
            # queues descriptor-stalled at ~130GB/s vs ~390GB/s burst.
            # nf leads on sync with clean 4KB descriptors; wee (tiny 96B
            # descriptors that clog the queue head) follows -- its f32 bias
            # cols are only needed by the SE chain at ~13us
            nc.sync.dma_start(out=nf_sb[:, 0:16, :], in_=nf[:, 0:16, :])
            nc.scalar.dma_start(out=wb_sb[:], in_=wb[:])
            # R-half in two chunks: octets 16..23 run while 24..32 streams
            # (the PE is idle there), shrinking the post-data chain
            nc.scalar.dma_start(out=nf_sb[:, 16:24, :], in_=nf[:, 16:24, :])
            nc.scalar.dma_start(out=nf_sb[:, 24:32, :], in_=nf[:, 24:32, :])
            nc.sync.dma_start(out=wee_sb[:], in_=wee[:])
            nc.sync.dma_start(out=wrow_sb[:], in_=wrow[:])

            # ---- PE warm-up + early SE act-table load during DMA wait ----
            scratch = constp.tile([128, 8], bf16, tag="scratch")
            nc.vector.memset(scratch[:], 0.0)
            scratch2 = constp.tile([128, 8], bf16, tag="scratch2")
            nc.scalar.activation(scratch2[:], scratch[:], Act.Relu)
            warm_rhs = constp.tile([128, 256], bf16, tag="warm_rhs")
            nc.vector.memset(warm_rhs[:], 0.0)
            ph = php.tile([128, 256], f32, tag="ph")

            pwj = pcp.tile([128, 256], f32, tag="pc", name="pwj")

            def warm(n):
                # wide junk matmuls (N=256 keeps the PE array dense enough to
                # keep the HAM un-throttled); target is a pc-pool PSUM scratch
                # slot that the pc matmuls overwrite later (same-engine order)
                for _ in range(n):
                    nc.tensor.matmul(
                        pwj[:, 0:256], lhsT=warm_rhs[:, 0:128], rhs=warm_rhs[:],
                        start=True, stop=True,
                    )

            warm(24)

            ones_sb = constp.tile([1, 256], bf16, tag="ones")
            nc.vector.memset(ones_sb[:], 1.0)

            # ---- hT[f, j]: all 32 octets in arrival order; everything
            # downstream (pa/pcl/pcr, casts) pipelines behind the data ----
            cTall = datap.tile([128, 2, 256], bf16, tag="cTall")
            cT = [cTall[:, t, :] for t in range(2)]
            aTb4 = [
                datap.tile([128, 128, 4], f32, tag=f"aTb4{t}", name=f"aTb4{t}")
                for t in range(2)
            ]
            negA2 = [
                datap.tile([128, 128, 2], bf16, tag=f"negA2{t}", name=f"negA2{t}")
                for t in range(2)
            ]
            # each hT cast is emitted right after its own octet group so its
            # semaphore wait stays fine-grained (emitting casts after ALL
            # octets coarsens the wait to the full 32-octet count)
            hTb = datap.tile([128, 256], bf16, tag="hTb")
            for olist, j0, jw in (
                (range(0, 16), 0, 128),
                (range(16, 32), 128, 128),
            ):
                for o in olist:
                    nc.tensor.matmul(
                        ph[:, 8 * o : 8 * o + 8],
                        lhsT=nf_sb[:, o, :],
                        rhs=wb_sb[:, :, _WB_SM : _WB_SM + 4],
                        start=True,
                        stop=True,
                    )
                nc.vector.tensor_copy(hTb[:, j0 : j0 + jw], ph[:, j0 : j0 + jw])
            pas = []
            for t in range(2):
                pa = pap.tile([128, 128], f32, tag="pa", name=f"pa{t}")
                nc.tensor.matmul(
                    pa[:], lhsT=wb_sb[:, t, _WB_A : _WB_A + 128], rhs=hTb[:, 0:128],
                    start=True, stop=True,
                )
                pas.append(pa)
                # negA2 = -(a + b1), written directly in pair-duplicated
                # form (innermost dim stride-1 pairs for the act TT in1);
                # first on SE -- the TT path is critical
                nc.scalar.activation(
                    negA2[t][:, :, :],
                    pa[:].unsqueeze(2).broadcast_to([128, 128, 2]),
                    Act.Identity,
                    bias=wee_sb[:, _WEE_NB1T0 + 4 * t : _WEE_NB1T0 + 4 * t + 1],
                    scale=-1.0,
                )
            # cT via paired PSUM tiles: both t-halves drain in ONE cast
            for j0, jw in ((0, 128), (128, 128)):
                pcp_t = pcp.tile([128, 2, jw], f32, tag="pc", name=f"pc{j0}")
                for t in range(2):
                    nc.tensor.matmul(
                        pcp_t[:, t, :],
                        lhsT=wb_sb[:, t, _WB_C : _WB_C + 128],
                        rhs=hTb[:, j0 : j0 + jw],
                        start=True,
                        stop=True,
                    )
                nc.vector.tensor_copy(cTall[:, :, j0 : j0 + jw], pcp_t[:])
            for t in (1, 0):
                # t=1 first: superblock 0's SE singles are t=1 tiles
                nc.scalar.activation(
                    aTb4[t][:, :, :],
                    pas[t][:].broadcast_to([128, 128, 4]),
                    Act.Identity,
                    bias=wee_sb[:, _WEE_B1T0 + 4 * t : _WEE_B1T0 + 4 * t + 1],
                )

            # ---- b2u row: b2 + sum_t mask_t * (w2_t . A_t), on-device ----
            # pv matmuls here (cheap, on PE); the VE part of the chain is
            # emitted inside the act loop after superblock 0 so it does not
            # delay the first act TT in VE program order.
            pvs = []
            for t in range(2):
                pv = pap.tile([1, 256], f32, tag="pa", name=f"pv{t}")
                # pv[0,2i(+1)] = sum_p w2_t[p] * negA2_t[p,i]  (= -(w2.A_i)_t)
                nc.tensor.matmul(
                    pv[:],
                    lhsT=wb_sb[:, t, _WB_W2 + 31 : _WB_W2 + 32],
                    rhs=negA2[t][:, :, :],
                    start=True,
                    stop=True,
                )
                pvs.append(pv)
            b2u = datap.tile([1, 128], bf16, tag="b2u")
            b2uT = datap.tile([32, 4], f32, tag="b2uT")

            def emit_b2u_chain():
                vmt = []
                for t in range(2):
                    vr = datap.tile([1, 128], f32, tag=f"vr{t}")
                    nc.vector.tensor_copy(vr[:], pvs[t][:, 0:256:2])
                    vm = datap.tile([1, 128], f32, tag=f"vm{t}")
                    # wrow mask cols hold -mask_t: vm = mask_t * (w2 . A)_t
                    nc.vector.tensor_tensor(
                        vm[:], vr[:],
                        wrow_sb[:, _WROW_M0 + 128 * t : _WROW_M0 + 128 * t + 128],
                        Alu.mult,
                    )
                    vmt.append(vm)
                vsum = datap.tile([1, 128], f32, tag="vsum")
                nc.vector.tensor_tensor(vsum[:], vmt[0][:], vmt[1][:], Alu.add)
                b2u_f = datap.tile([1, 128], f32, tag="b2u_f")
                nc.vector.tensor_tensor(
                    b2u_f[:], vsum[:], wrow_sb[:, _WROW_B2 : _WROW_B2 + 128],
                    Alu.add,
                )
                nc.vector.tensor_copy(b2u[:], b2u_f[:])
                # transpose b2u to per-partition form [32 rows, 4 groups]
                # so the PSUM drains can fold the row-add in (no closing
                # matmul on the group-drain critical path)
                pb2t = pap.tile([32, 4], f32, tag="pa", name="pb2t")
                for gg in range(4):
                    nc.tensor.matmul(
                        pb2t[:, gg : gg + 1],
                        lhsT=b2u[0:1, 32 * gg : 32 * gg + 32],
                        rhs=ones_sb[0:1, 0:1],
                        start=True,
                        stop=True,
                    )
                nc.vector.tensor_copy(b2uT[:], pb2t[:])

            # ---- pairwise: act tiles on VE (TT blocks) + SE (singles) ----
            # stationary for (t, r): wb[:, t, W2+31-r : W2+63-r]  (col r = w2_t)
            for g in range(4):
                pl = plp.tile([32, 256], f32, tag="pl", name=f"pl{g}")
                for sbl in range(4):
                    sb = 4 * g + sbl
                    i0 = _SBW * sb
                    mv = {}  # (i, t) -> rhs AP for the matmul
                    # SE singles first (SE production runs ahead of the PE
                    # so a just-in-time single never stalls it)
                    for k in range(_SBW):
                        for t in range(2):
                            if k < _PLAN[sb][t]:
                                continue
                            asb = actp.tile([128, 256], bf16, tag="acts", bufs=16)
                            nc.scalar.activation(
                                asb[:], cT[t], Act.Relu,
                                bias=aTb4[t][:, i0 + k, 0:1],
                            )
                            mv[(i0 + k, t)] = asb[:]
                    # VE TT blocks.  The first superblock is split into small
                    # t-INTERLEAVED chunks: the PE consumes (i, t0), (i, t1),
                    # ... so producing all of t0 before any t1 would stall
                    # its second matmul for a full block.
                    if sb == 0:
                        chunk_ends = [2, 4, _SBW]
                    else:
                        chunk_ends = [_SBW]
                    prev = 0
                    for c1e in chunk_ends:
                        for t in range(2):
                            nb = _PLAN[sb][t]
                            c0 = min(prev, nb)
                            c1 = min(c1e, nb)
                            w = c1 - c0
                            if w <= 0:
                                continue
                            av = actp.tile(
                                [128, w, 256], bf16, tag=f"av{w}", bufs=10
                            )
                            nc.vector.tensor_tensor(
                                av[:],
                                cT[t].unsqueeze(1).broadcast_to([128, w, 256]),
                                negA2[t][:, i0 + c0 : i0 + c1, :]
                                .unsqueeze(2)
                                .broadcast_to([128, w, 128, 2]),
                                Alu.max,
                            )
                            for k in range(w):
                                mv[(i0 + c0 + k, t)] = av[:, k, :]
                        prev = c1e
                    # matmuls in consumption order; the group's last act
                    # matmul closes the accumulation (b2u is added by the
                    # drain itself, not a closing matmul)
                    for k in range(_SBW):
                        r = _SBW * sbl + k
                        for t in range(2):
                            nc.tensor.matmul(
                                pl[:, :],
                                lhsT=wb_sb[:, t, _WB_W2 + 31 - r : _WB_W2 + 63 - r],
                                rhs=mv[(i0 + k, t)],
                                start=(r == 0 and t == 0),
                                stop=(r == 31 and t == 1),
                            )
                    if sb == 0:
                        # virtual-time floor stops the Tile scheduler from
                        # front-running these VE ops ahead of the critical
                        # first act TTs (b2u is only needed at group close)
                        with tc.tile_wait_until(0.018):
                            emit_b2u_chain()
                # drain with the b2u row-add folded in (exact f32 add)
                osb = datap.tile([32, 256], f32, tag=f"osb{g}", name=f"osb{g}")
                if _DRAINS[g] == "v":
                    nc.vector.tensor_scalar(
                        osb[:], pl[:, :], b2uT[:, g : g + 1], None, Alu.add
                    )
                else:
                    nc.scalar.activation(
                        osb[:], pl[:, :], Act.Identity, bias=b2uT[:, g : g + 1]
                    )
                nc.sync.dma_start(out=outd[g], in_=osb[:])

    nc.compile()
    return nc


def make_in_maps(nodefeat, W1, b1, W2, b2):
    """Host-side sharding/layout prep."""
    import ml_dtypes

    bf16 = ml_dtypes.bfloat16
    nodefeat = np.asarray(nodefeat, dtype=np.float32)
    W1 = np.asarray(W1, dtype=np.float32)
    b1 = np.asarray(b1, dtype=np.float32)
    W2 = np.asarray(W2, dtype=np.float32)
    b2 = np.asarray(b2, dtype=np.float32)

    wee = np.zeros((128, _WEE_W), dtype=np.float32)
    wee[:, _WEE_SMAT : _WEE_SMAT + 8] = (
        np.repeat(np.eye(8, dtype=np.float32), 16, axis=0) / 16.0
    )
    b1t = b1.reshape(2, 128).T  # [p, t]
    wee[:, _WEE_B1T0] = b1t[:, 0]
    wee[:, _WEE_B1T1] = b1t[:, 1]
    wee[:, _WEE_NB1T0] = -b1t[:, 0]
    wee[:, _WEE_NB1T1] = -b1t[:, 1]

    wrow = np.zeros((1, _WROW_W), dtype=np.float32)
    wrow[0, _WROW_M0 : _WROW_M0 + 128] = -_MASK[0]
    wrow[0, _WROW_M1 : _WROW_M1 + 128] = -_MASK[1]
    wrow[0, _WROW_B2 : _WROW_B2 + 128] = b2[0]

    W1a, W1c = W1[:, :_F], W1[:, _F:]
    w2r = W2[0].reshape(2, 128)  # [t, p]
    wbp = np.zeros((128, 2, _WB_W), dtype=np.float32)
    wbp[:, 0, _WB_A : _WB_A + 128] = W1a[:128].T
    wbp[:, 1, _WB_A : _WB_A + 128] = W1a[128:].T
    wbp[:, 0, _WB_C : _WB_C + 128] = W1c[:128].T
    wbp[:, 1, _WB_C : _WB_C + 128] = W1c[128:].T
    wbp[:, :, _WB_W2 + 31] = w2r.T
    smat = np.repeat(np.eye(8, dtype=np.float32), 16, axis=0) / 16.0
    wbp[:, 0, _WB_SM : _WB_SM + 4] = smat[:, 0:4]
    wbp[:, 1, _WB_SM : _WB_SM + 4] = smat[:, 4:8]
    wbp = np.ascontiguousarray(wbp.astype(bf16))

    in_maps = []
    for k in range(_NCORES):
        b, ih = divmod(k, 2)
        nf_b = nodefeat[b]  # [256, 16, 128]
        if ih:
            nf_b = np.concatenate([nf_b[128:], nf_b[:128]], axis=0)
        # [256,16,128] -> [32 oct, (j8,t16)=128, 128 f] -> [128, 32, 128]
        nf_dev = np.ascontiguousarray(
            nf_b.reshape(32, 128, 128).transpose(1, 0, 2).astype(bf16)
        )
        in_maps.append({"nf": nf_dev, "wee": wee, "wrow": wrow, "wb": wbp})
    return in_maps


def assemble_output(results):
    out = np.empty((_B, _N, _N), dtype=np.float32)
    for k in range(_NCORES):
        b, ih = divmod(k, 2)
        r = results[k]["out"].reshape(128, 256)  # [i, j] (j core-local order)
        if ih:
            r = np.concatenate([r[:, 128:], r[:, :128]], axis=1)
        out[b, ih * 128 : (ih + 1) * 128, :] = r
    return out


def _get_nc():
    if "nc" not in _CACHE:
        _CACHE["nc"] = build_nc()
    return _CACHE["nc"]


def kernel(nodefeat, W1, b1, W2, b2):
    _ensure_paths()
    from concourse.bass_utils import run_bass_kernel_spmd

    nc = _get_nc()
    in_maps = make_in_maps(nodefeat, W1, b1, W2, b2)
    res = run_bass_kernel_spmd(nc, in_maps, list(range(_NCORES)))
    return assemble_output(res.results)


# revision 59
# speedup vs baseline: 1.0740x; 1.0740x over previous
"""Trainium2 Bass kernel for nn_LinkPredictor.

Reference computation (B=4, N=256, T=16, F=128, H=256):
    h = mean_T(nodefeat)                      # [B,N,F]
    a = h @ W1[:, :F].T                       # [B,N,H]
    c = h @ W1[:, F:].T                       # [B,N,H]
    logits[b,i,j] = W2[0] . relu(a[b,i] + c[b,j] + b1) + b2   # [B,N,N]

Sharding: 8 cores; core k handles batch b=k//2, i-half k%2 (128 i-rows x
256 j-cols of one batch's NxN grid).  Each core only needs nodefeat[b].

v8 design (52.7us measured healthy vs 56.4-57.5us v5.3 baseline; known
run-to-run variance ±1.4us from Tile compile-schedule nondeterminism,
and up to +20% when the chip sits in the thermal P-state -- HAM event
duration 4095ns instead of 3413ns.  Timeline at 52.8us: data in by
~13.2 (two 512KB nf DMAs, 4KB descriptors), first act TT ~15.5, act
phase PE-bound at ~109ns/matmul ending ~48, then drain + a fixed ~10us
framework postamble of 255 semaphore clears):
  - The 256 act tiles [128h, 256j] are produced by VE + SE in parallel.
    HW microbench: DVE tensor_scalar with a per-partition scalar POINTER
    is locked to 2x_1p (196ns/tile) -- the ptr occupies the second read
    port -- while immediates reach 4x.  Instead VE produces act tiles in
    BATCHES via tensor_tensor MAX using relu(c+A) = max(c, -A) + A:
    one TT covers nb consecutive i's of one t-half (in0 = cT broadcast
    over i via a stride-0 dim; in1 = pair-duplicated negA2 with a
    stride-1 innermost dim to keep the 2x_1p mode; ~150ns/tile at nb=6).
    The missing + w2.A_i term is a per-row scalar folded into the b2u
    row: b2u[i] = b2 + sum_t mask_t[i] * (w2_t . A_ti), with (w2_t.A_t)
    computed by two 1-col matmuls, masked by a host-baked -mask row
    (VE-covered (i,t) only), then transposed to per-partition form by
    four 1-col matmuls.  The PSUM DRAIN adds it (VE tensor_scalar add /
    SE activation bias, exact f32) -- no closing matmul sits on the
    group-drain critical path; the last act matmul carries stop=True.
  - SE covers the per-block remainder (8-nb tiles) as Relu+bias singles
    from aTb4; a static greedy balances VE/SE clocks (nb ~ 6) including
    the 4 PSUM group drains, which alternate between VE and SE.
  - act loop is r-major/t-inner; per 8-i superblock: TT blocks first,
    then SE singles, then the 16 matmuls in consumption order.  PE is
    the bottleneck at ~109ns/matmul -> ~28.5us pairwise phase.
  - weights packed into one bf16 tensor wb=[w1a|w1c|w2pad] (1 DMA);
    nf split 2+2 across the sync/scalar HWDGE queues; wee/wrow first.
  - cT is built per j-half (pcL from octets 0..15, pcR from 16..31) so
    the pairwise phase starts ~1.5us after the last nf octet lands.
  - wide full-array dummy matmuls during the DMA wait pre-trip the HAM
    clock gate; SE act-table preloaded by a scratch Relu.
"""

import os
import sys

import numpy as np

_B, _N, _T, _F, _H = 4, 256, 16, 128, 256
_NCORES = 8

_CACHE = {}


def _ensure_paths():
    for p in (
        "/root/.axon_site",
        "/root/.axon_site/_ro/trn_rl_repo",
        "/root/.axon_site/_ro/pypackages",
        "/opt/trn_rl_repo",
    ):
        if os.path.isdir(p) and p not in sys.path:
            sys.path.append(p)


# wee param layout (f32 columns; scalar cols 16B-aligned)
_WEE_SMAT = 0   # [0:8) smat
_WEE_B1T0 = 8   # b1 (t=0)
_WEE_B1T1 = 12  # b1 (t=1)
_WEE_NB1T0 = 16  # -b1 (t=0)
_WEE_NB1T1 = 20  # -b1 (t=1)
_WEE_W = 24

# wrow [1, 384] f32: [0:128)=-mask_t0, [128:256)=-mask_t1, [256:384)=b2
_WROW_M0 = 0
_WROW_M1 = 128
_WROW_B2 = 256
_WROW_W = 384

# wb packed weights [128, 2, 323]: [0:128)=w1a.T, [128:256)=w1c.T,
# [256:319)=w2 sliding-window pad (center col 31), [319:323)=smat half
# (t=0 cols hold smat[:,0:4], t=1 cols smat[:,4:8] -- the octet matmul
# streams rhs wb[:, :, 319:323] as 8 cols in j-local order)
_WB_A = 0
_WB_C = 128
_WB_W2 = 256
_WB_SM = 319
_WB_W = 323

_CS = 414.0        # SE activation single [128,256]
_CDRAIN = 400.0    # pl->osb drain


def _tt_cost(nb):
    # VE tensor_tensor bf16 2x_1p, FD = 256*nb
    return (58.0 + 128.0 * nb) / 0.96


_SBW = 8  # superblock width (i's per VE/SE scheduling block)


def _mk_sched():
    """Greedy VE/SE split over 8-i superblocks.  plan[sb][t] = nb
    (leading i's on VE via one TT; the remaining 8-nb are SE singles);
    drains[g] in {'v','s'}.  (16-i blocks with TT12 measured WORSE --
    coarser production granularity beats the per-tile saving.)  SE
    starts with a virtual handicap: its head chain (negA2/aTb4 builds)
    ends ~1.2us after VE is ready, so early superblocks lean on VE."""
    vclk, sclk = 0.0, 1200.0
    plan = [[0, 0] for _ in range(16)]
    drains = []
    for sb in range(16):
        for t in range(2):
            best = None
            for nb in range(_SBW + 1):
                v2 = vclk + (_tt_cost(nb) if nb else 0.0)
                s2 = sclk + _CS * (_SBW - nb)
                m = max(v2, s2)
                if best is None or m < best[0]:
                    best = (m, nb, v2, s2)
            _, nb, vclk, sclk = best
            plan[sb][t] = nb
        if sb % 4 == 3:
            if vclk <= sclk:
                vclk += _CDRAIN
                drains.append("v")
            else:
                sclk += _CDRAIN
                drains.append("s")
    return plan, drains


_PLAN, _DRAINS = _mk_sched()
# mask[t][i] = 1.0 if (i, t) produced via max-form (VE) -> needs +w2.A_i
_MASK = np.zeros((2, 128), dtype=np.float32)
for _sb in range(16):
    for _t in range(2):
        _MASK[_t, _SBW * _sb : _SBW * _sb + _PLAN[_sb][_t]] = 1.0


def build_nc():
    """Build the per-core Bass program (same program for all 8 cores)."""
    _ensure_paths()
    import concourse.mybir as mybir
    import concourse.tile as tile
    from concourse import bacc

    f32 = mybir.dt.float32
    bf16 = mybir.dt.bfloat16
    Alu = mybir.AluOpType
    Act = mybir.ActivationFunctionType

    nc = bacc.Bacc("TRN2", target_bir_lowering=False, debug=False)

    nf = nc.declare_dram_parameter("nf", [128, 32, 128], bf16, isOutput=False)
    wee = nc.declare_dram_parameter("wee", [128, _WEE_W], f32, isOutput=False)
    wrow = nc.declare_dram_parameter("wrow", [1, _WROW_W], f32, isOutput=False)
    wb = nc.declare_dram_parameter("wb", [128, 2, _WB_W], bf16, isOutput=False)
    outd = nc.declare_dram_parameter("out", [4, 32, 256], f32, isOutput=True)

    with tile.TileContext(nc) as tc:
        with (
            tc.tile_pool(name="const", bufs=1) as constp,
            tc.tile_pool(name="data", bufs=1) as datap,
            tc.tile_pool(name="act", bufs=10) as actp,
            tc.tile_pool(name="ph", bufs=1, space="PSUM") as php,
            tc.tile_pool(name="pa", bufs=2, space="PSUM") as pap,
            tc.tile_pool(name="pc", bufs=2, space="PSUM") as pcp,
            tc.tile_pool(name="pl", bufs=3, space="PSUM") as plp,
        ):
            # ---- DMAs: small params first, nf split across both queues ----
            nf_sb = constp.tile([128, 32, 128], bf16, tag="nf")
            wee_sb = constp.tile([128, _WEE_W], f32, tag="wee")
            wrow_sb = constp.tile([1, _WROW_W], f32, tag="wrow")
            wb_sb = constp.tile([128, 2, _WB_W], bf16, tag="wb")
            # The Scalar engine's queue starts ~2.9us late: its auto-inserted
            # ACT_TABLE_LOAD is a multi-us DMA serialized at the queue head.
            # So the early-critical path (wee for smat, octets 0..15) rides
            # the sync queue, and scalar only carries data needed after ~12us
            # (wb, the R-half octets).  nf moves as two 512KB DMAs (4KB
            # contiguous per partition): the 2KB-descriptor version left the
            # queues descriptor-stalled at ~130GB/s vs ~390GB/s burst.
            # nf leads on sync with clean 4KB descriptors; wee (tiny 96B
            # descriptors that clog the queue head) follows -- its f32 bias
            # cols are only needed by the SE chain at ~13us
            nc.sync.dma_start(out=nf_sb[:, 0:16, :], in_=nf[:, 0:16, :])
            nc.scalar.dma_start(out=wb_sb[:], in_=wb[:])
            nc.scalar.dma_start(out=nf_sb[:, 16:32, :], in_=nf[:, 16:32, :])
            nc.sync.dma_start(out=wee_sb[:], in_=wee[:])
            nc.sync.dma_start(out=wrow_sb[:], in_=wrow[:])

            # ---- PE warm-up + early SE act-table load during DMA wait ----
            scratch = constp.tile([128, 8], bf16, tag="scratch")
            nc.vector.memset(scratch[:], 0.0)
            scratch2 = constp.tile([128, 8], bf16, tag="scratch2")
            nc.scalar.activation(scratch2[:], scratch[:], Act.Relu)
            warm_rhs = constp.tile([128, 256], bf16, tag="warm_rhs")
            nc.vector.memset(warm_rhs[:], 0.0)
            ph = php.tile([128, 256], f32, tag="ph")

            pwj = pcp.tile([128, 256], f32, tag="pc", name="pwj")

            def warm(n):
                # wide junk matmuls (N=256 keeps the PE array dense enough to
                # keep the HAM un-throttled); target is a pc-pool PSUM scratch
                # slot that the pc matmuls overwrite later (same-engine order)
                for _ in range(n):
                    nc.tensor.matmul(
                        pwj[:, 0:256], lhsT=warm_rhs[:, 0:128], rhs=warm_rhs[:],
                        start=True, stop=True,
                    )

            warm(24)

            ones_sb = constp.tile([1, 256], bf16, tag="ones")
            nc.vector.memset(ones_sb[:], 1.0)

            # ---- hT[f, j]: all 32 octets in arrival order; everything
            # downstream (pa/pcl/pcr, casts) pipelines behind the data ----
            cTall = datap.tile([128, 2, 256], bf16, tag="cTall")
            cT = [cTall[:, t, :] for t in range(2)]
            aTb4 = [
                datap.tile([128, 128, 4], f32, tag=f"aTb4{t}", name=f"aTb4{t}")
                for t in range(2)
            ]
            negA2 = [
                datap.tile([128, 128, 2], bf16, tag=f"negA2{t}", name=f"negA2{t}")
                for t in range(2)
            ]
            # each hT cast is emitted right after its own octet group so its
            # semaphore wait stays fine-grained (emitting casts after ALL
            # octets coarsens the wait to the full 32-octet count)
            hTb = datap.tile([128, 256], bf16, tag="hTb")

            def octet_group(olist, j0, jw):
                for o in olist:
                    nc.tensor.matmul(
                        ph[:, 8 * o : 8 * o + 8],
                        lhsT=nf_sb[:, o, :],
                        rhs=wb_sb[:, :, _WB_SM : _WB_SM + 4],
                        start=True,
                        stop=True,
                    )
                nc.vector.tensor_copy(hTb[:, j0 : j0 + jw], ph[:, j0 : j0 + jw])

            def pc_pair(j0, jw):
                # paired PSUM tile: both t-halves of cT drain in ONE cast
                pcp_t = pcp.tile([128, 2, jw], f32, tag="pc", name=f"pc{j0}")
                for t in range(2):
                    nc.tensor.matmul(
                        pcp_t[:, t, :],
                        lhsT=wb_sb[:, t, _WB_C : _WB_C + 128],
                        rhs=hTb[:, j0 : j0 + jw],
                        start=True,
                        stop=True,
                    )
                nc.vector.tensor_copy(cTall[:, :, j0 : j0 + jw], pcp_t[:])

            octet_group(range(0, 16), 0, 128)
            # The ENTIRE L-side chain (pa/pcl + negA2 on SE) is emitted
            # BEFORE the R-octets: the in-order PE would otherwise block on
            # the R-half DMA (~13.2us) with pa/pcl runnable at ~12.2us.
            pas = []
            for t in range(2):
                pa = pap.tile([128, 128], f32, tag="pa", name=f"pa{t}")
                nc.tensor.matmul(
                    pa[:], lhsT=wb_sb[:, t, _WB_A : _WB_A + 128], rhs=hTb[:, 0:128],
                    start=True, stop=True,
                )
                pas.append(pa)
                # negA2 = -(a + b1), written directly in pair-duplicated
                # form (innermost dim stride-1 pairs for the act TT in1);
                # first on SE -- the TT path is critical
                nc.scalar.activation(
                    negA2[t][:, :, :],
                    pa[:].unsqueeze(2).broadcast_to([128, 128, 2]),
                    Act.Identity,
                    bias=wee_sb[:, _WEE_NB1T0 + 4 * t : _WEE_NB1T0 + 4 * t + 1],
                    scale=-1.0,
                )
            pc_pair(0, 128)
            octet_group(range(16, 32), 128, 128)
            pc_pair(128, 128)
            for t in (1, 0):
                # t=1 first: superblock 0's SE singles are t=1 tiles
                nc.scalar.activation(
                    aTb4[t][:, :, :],
                    pas[t][:].broadcast_to([128, 128, 4]),
                    Act.Identity,
                    bias=wee_sb[:, _WEE_B1T0 + 4 * t : _WEE_B1T0 + 4 * t + 1],
                )

            # ---- b2u row: b2 + sum_t mask_t * (w2_t . A_t), on-device ----
            # pv matmuls here (cheap, on PE); the VE part of the chain is
            # emitted inside the act loop after superblock 0 so it does not
            # delay the first act TT in VE program order.
            pvs = []
            for t in range(2):
                pv = pap.tile([1, 256], f32, tag="pa", name=f"pv{t}")
                # pv[0,2i(+1)] = sum_p w2_t[p] * negA2_t[p,i]  (= -(w2.A_i)_t)
                nc.tensor.matmul(
                    pv[:],
                    lhsT=wb_sb[:, t, _WB_W2 + 31 : _WB_W2 + 32],
                    rhs=negA2[t][:, :, :],
                    start=True,
                    stop=True,
                )
                pvs.append(pv)
            b2u = datap.tile([1, 128], bf16, tag="b2u")
            b2uT = datap.tile([32, 4], f32, tag="b2uT")

            def emit_b2u_chain():
                vmt = []
                for t in range(2):
                    vr = datap.tile([1, 128], f32, tag=f"vr{t}")
                    nc.vector.tensor_copy(vr[:], pvs[t][:, 0:256:2])
                    vm = datap.tile([1, 128], f32, tag=f"vm{t}")
                    # wrow mask cols hold -mask_t: vm = mask_t * (w2 . A)_t
                    nc.vector.tensor_tensor(
                        vm[:], vr[:],
                        wrow_sb[:, _WROW_M0 + 128 * t : _WROW_M0 + 128 * t + 128],
                        Alu.mult,
                    )
                    vmt.append(vm)
                vsum = datap.tile([1, 128], f32, tag="vsum")
                nc.vector.tensor_tensor(vsum[:], vmt[0][:], vmt[1][:], Alu.add)
                b2u_f = datap.tile([1, 128], f32, tag="b2u_f")
                nc.vector.tensor_tensor(
                    b2u_f[:], vsum[:], wrow_sb[:, _WROW_B2 : _WROW_B2 + 128],
                    Alu.add,
                )
                nc.vector.tensor_copy(b2u[:], b2u_f[:])
                # transpose b2u to per-partition form [32 rows, 4 groups]
                # so the PSUM drains can fold the row-add in (no closing
                # matmul on the group-drain critical path)
                pb2t = pap.tile([32, 4], f32, tag="pa", name="pb2t")
                for gg in range(4):
                    nc.tensor.matmul(
                        pb2t[:, gg : gg + 1],
                        lhsT=b2u[0:1, 32 * gg : 32 * gg + 32],
                        rhs=ones_sb[0:1, 0:1],
                        start=True,
                        stop=True,
                    )
                nc.vector.tensor_copy(b2uT[:], pb2t[:])

            # ---- pairwise: act tiles on VE (TT blocks) + SE (singles) ----
            # stationary for (t, r): wb[:, t, W2+31-r : W2+63-r]  (col r = w2_t)
            for g in range(4):
                pl = plp.tile([32, 256], f32, tag="pl", name=f"pl{g}")
                for sbl in range(4):
                    sb = 4 * g + sbl
                    i0 = _SBW * sb
                    mv = {}  # (i, t) -> rhs AP for the matmul
                    # SE singles first (SE production runs ahead of the PE
                    # so a just-in-time single never stalls it)
                    for k in range(_SBW):
                        for t in range(2):
                            if k < _PLAN[sb][t]:
                                continue
                            asb = actp.tile([128, 256], bf16, tag="acts", bufs=12)
                            nc.scalar.activation(
                                asb[:], cT[t], Act.Relu,
                                bias=aTb4[t][:, i0 + k, 0:1],
                            )
                            mv[(i0 + k, t)] = asb[:]
                    # VE TT blocks.  The first superblock is split into small
                    # t-INTERLEAVED chunks: the PE consumes (i, t0), (i, t1),
                    # ... so producing all of t0 before any t1 would stall
                    # its second matmul for a full block.
                    if sb == 0:
                        chunk_ends = [2, 4, _SBW]
                    else:
                        chunk_ends = [_SBW]
                    prev = 0
                    for c1e in chunk_ends:
                        for t in range(2):
                            nb = _PLAN[sb][t]
                            c0 = min(prev, nb)
                            c1 = min(c1e, nb)
                            w = c1 - c0
                            if w <= 0:
                                continue
                            av = actp.tile(
                                [128, w, 256], bf16, tag=f"av{w}", bufs=6
                            )
                            nc.vector.tensor_tensor(
                                av[:],
                                cT[t].unsqueeze(1).broadcast_to([128, w, 256]),
                                negA2[t][:, i0 + c0 : i0 + c1, :]
                                .unsqueeze(2)
                                .broadcast_to([128, w, 128, 2]),
                                Alu.max,
                            )
                            for k in range(w):
                                mv[(i0 + c0 + k, t)] = av[:, k, :]
                        prev = c1e
                    # matmuls in consumption order; the group's last act
                    # matmul closes the accumulation (b2u is added by the
                    # drain itself, not a closing matmul)
                    for k in range(_SBW):
                        r = _SBW * sbl + k
                        for t in range(2):
                            nc.tensor.matmul(
                                pl[:, :],
                                lhsT=wb_sb[:, t, _WB_W2 + 31 - r : _WB_W2 + 63 - r],
                                rhs=mv[(i0 + k, t)],
                                start=(r == 0 and t == 0),
                                stop=(r == 31 and t == 1),
                            )
                    if sb == 0:
                        # virtual-time floor stops the Tile scheduler from
                        # front-running these VE ops ahead of the critical
                        # first act TTs (b2u is only needed at group close)
                        with tc.tile_wait_until(0.018):
                            emit_b2u_chain()
                # drain with the b2u row-add folded in (exact f32 add)
                osb = datap.tile([32, 256], f32, tag=f"osb{g}", name=f"osb{g}")
                if _DRAINS[g] == "v":
                    nc.vector.tensor_scalar(
                        osb[:], pl[:, :], b2uT[:, g : g + 1], None, Alu.add
                    )
                else:
                    nc.scalar.activation(
                        osb[:], pl[:, :], Act.Identity, bias=b2uT[:, g : g + 1]
                    )
                nc.sync.dma_start(out=outd[g], in_=osb[:])

    nc.compile()
    return nc


def make_in_maps(nodefeat, W1, b1, W2, b2):
    """Host-side sharding/layout prep."""
    import ml_dtypes

    bf16 = ml_dtypes.bfloat16
    nodefeat = np.asarray(nodefeat, dtype=np.float32)
    W1 = np.asarray(W1, dtype=np.float32)
    b1 = np.asarray(b1, dtype=np.float32)
    W2 = np.asarray(W2, dtype=np.float32)
    b2 = np.asarray(b2, dtype=np.float32)

    wee = np.zeros((128, _WEE_W), dtype=np.float32)
    wee[:, _WEE_SMAT : _WEE_SMAT + 8] = (
        np.repeat(np.eye(8, dtype=np.float32), 16, axis=0) / 16.0
    )
    b1t = b1.reshape(2, 128).T  # [p, t]
    wee[:, _WEE_B1T0] = b1t[:, 0]
    wee[:, _WEE_B1T1] = b1t[:, 1]
    wee[:, _WEE_NB1T0] = -b1t[:, 0]
    wee[:, _WEE_NB1T1] = -b1t[:, 1]

    wrow = np.zeros((1, _WROW_W), dtype=np.float32)
    wrow[0, _WROW_M0 : _WROW_M0 + 128] = -_MASK[0]
    wrow[0, _WROW_M1 : _WROW_M1 + 128] = -_MASK[1]
    wrow[0, _WROW_B2 : _WROW_B2 + 128] = b2[0]

    W1a, W1c = W1[:, :_F], W1[:, _F:]
    w2r = W2[0].reshape(2, 128)  # [t, p]
    wbp = np.zeros((128, 2, _WB_W), dtype=np.float32)
    wbp[:, 0, _WB_A : _WB_A + 128] = W1a[:128].T
    wbp[:, 1, _WB_A : _WB_A + 128] = W1a[128:].T
    wbp[:, 0, _WB_C : _WB_C + 128] = W1c[:128].T
    wbp[:, 1, _WB_C : _WB_C + 128] = W1c[128:].T
    wbp[:, :, _WB_W2 + 31] = w2r.T
    smat = np.repeat(np.eye(8, dtype=np.float32), 16, axis=0) / 16.0
    wbp[:, 0, _WB_SM : _WB_SM + 4] = smat[:, 0:4]
    wbp[:, 1, _WB_SM : _WB_SM + 4] = smat[:, 4:8]
    wbp = np.ascontiguousarray(wbp.astype(bf16))

    in_maps = []
    for k in range(_NCORES):
        b, ih = divmod(k, 2)
        nf_b = nodefeat[b]  # [256, 16, 128]
        if ih:
            nf_b = np.concatenate([nf_b[128:], nf_b[:128]], axis=0)
        # [256,16,128] -> [32 oct, (j8,t16)=128, 128 f] -> [128, 32, 128]
        nf_dev = np.ascontiguousarray(
            nf_b.reshape(32, 128, 128).transpose(1, 0, 2).astype(bf16)
        )
        in_maps.append({"nf": nf_dev, "wee": wee, "wrow": wrow, "wb": wbp})
    return in_maps


def assemble_output(results):
    out = np.empty((_B, _N, _N), dtype=np.float32)
    for k in range(_NCORES):
        b, ih = divmod(k, 2)
        r = results[k]["out"].reshape(128, 256)  # [i, j] (j core-local order)
        if ih:
            r = np.concatenate([r[:, 128:], r[:, :128]], axis=1)
        out[b, ih * 128 : (ih + 1) * 128, :] = r
    return out


def _get_nc():
    if "nc" not in _CACHE:
        _CACHE["nc"] = build_nc()
    return _CACHE["nc"]


def kernel(nodefeat, W1, b1, W2, b2):
    _ensure_paths()
    from concourse.bass_utils import run_bass_kernel_spmd

    nc = _get_nc()
    in_maps = make_in_maps(nodefeat, W1, b1, W2, b2)
    res = run_bass_kernel_spmd(nc, in_maps, list(range(_NCORES)))
    return assemble_output(res.results)
